# revision 4
# baseline (speedup 1.0000x reference)
"""Trainium2 Bass kernel: GQA multi-head self-attention (B=1, L=4096, D=1024,
16 Q heads, 4 KV heads, head_dim 64, interleaved RoPE, causal softmax).

Sharding: 2 query heads + their (shared) KV head per core, 8 cores.
Each core computes a full-shape partial output Y_c.T = (attn_c @ Wo_c.T).T
(Megatron row-parallel style); the host sums the 8 partials.

Design notes:
  - Scores run as S.T = K @ Q.T ([128 keys, 2 heads x 512 q] PSUM tiles); exp
    runs on the scalar engine straight out of PSUM; diagonal key blocks trim
    the causally-dead columns from both the matmul and the exp.
  - PV uses P as the stationary operand (full 128x128 array) streaming
    [V | 1] blocks, accumulating [q, d] tiles whose 65th column is the
    softmax denominator, so normalize is reciprocal + per-partition
    tensor_scalar multiply fused into the PSUM evacuation.
  - A PSUM zero region is 2KB: only the first matmul into each PV
    accumulator bank sets start=True; other slots' first writes consume the
    bank-wide pending-zero flag.
  - Output projection contracts both heads at once (K=128) after a PE
    transpose of the normalized attention output.
  - The projection pipeline for chunk qc+2 and the finish/out-projection of
    chunk qc-1 are sliced into small "filler" closures interleaved one per
    PV step, keeping the tensor engine fed between exp-paced score blocks.
  - Mask multiplies and SBUF-SBUF swap DMAs ride on gpsimd; big HBM loads
    split across SP/gpsimd/scalar queues; the tail chunk's finish alternates
    PSUM evacuations between the vector and scalar engines.
"""

import sys

for _p in ("/opt/trn_rl_repo",):
    if _p not in sys.path:
        sys.path.insert(0, _p)

import numpy as np

import concourse.bacc as bacc
import concourse.mybir as mybir
import concourse.tile as tile
from concourse.bass_utils import run_bass_kernel_spmd

F32 = mybir.dt.float32
F16 = mybir.dt.float16

D_MODEL = 1024
NUM_HEADS = 16
NUM_KV_HEADS = 4
HEAD_DIM = 64
THETA = 10000.0
N_CORES = 8
QC = 512          # query chunk
KB = 128          # key block


def build_kernel(L=4096):
    """One-core SPMD program. Handles its 2 query heads + 1 shared KV head."""
    nc = bacc.Bacc(None, target_bir_lowering=False)
    LC = L // QC          # number of 512-wide l/q chunks
    NT = L // KB          # number of 128-row key blocks / V tiles

    xt = nc.dram_tensor("xt", [D_MODEL, L], F16, kind="ExternalInput")
    wqt = nc.dram_tensor("wqt", [D_MODEL, 128], F16, kind="ExternalInput")
    wkvt = nc.dram_tensor("wkvt", [D_MODEL, 128], F16, kind="ExternalInput")
    wop = nc.dram_tensor("wop", [128, D_MODEL], F16, kind="ExternalInput")
    ctab = nc.dram_tensor("ctab", [128, L], F16, kind="ExternalInput")
    s3tab = nc.dram_tensor("s3tab", [128, L], F16, kind="ExternalInput")
    tri = nc.dram_tensor("tri", [128, 128], F16, kind="ExternalInput")
    identlo = nc.dram_tensor("identlo", [128, 64], F16, kind="ExternalInput")
    ident = nc.dram_tensor("ident", [128, 128], F16, kind="ExternalInput")
    yt = nc.dram_tensor("yt", [D_MODEL, L], F16, kind="ExternalOutput")

    with tile.TileContext(nc) as tc:
        with (
            tc.tile_pool(name="consts", bufs=1) as consts,
            tc.tile_pool(name="big", bufs=1) as big,
            tc.tile_pool(name="xin", bufs=4) as xin,
            tc.tile_pool(name="work", bufs=8) as work,
            tc.tile_pool(name="osp", bufs=16) as osp,
            tc.tile_pool(name="ptp", bufs=6) as ptp,
            tc.tile_pool(name="stp", bufs=2, space="PSUM") as stp,
            tc.tile_pool(name="otp", bufs=2, space="PSUM") as otp,
            tc.tile_pool(name="mp", bufs=2, space="PSUM") as mp,
        ):
            # ---- constants in SBUF ----
            wqt_s = consts.tile([128, 8, 128], F16, tag="wqt")
            wkvt_s = consts.tile([128, 8, 128], F16, tag="wkvt")
            wop_s = consts.tile([128, D_MODEL], F16, tag="wop")
            ctab_s = consts.tile([128, L], F16, tag="ctab")
            s3tab_s = consts.tile([128, L], F16, tag="s3tab")
            tri_s = consts.tile([128, 128], F16, tag="tri")
            identlo_s = consts.tile([128, 64], F16, tag="identlo")
            ident_s = consts.tile([128, 128], F16, tag="ident")

            # ---- persistent per-core activations ----
            qtrope = big.tile([128, L], F16, tag="qtrope")      # [2*64 halfsplit d, L]
            kt2 = big.tile([128, L], F16, tag="kt2")            # K.T duplicated twice
            vn = big.tile([128, NT * 65], F16, tag="vn")        # [V | 1] blocks

            xtiles = {}

            xt_r = xt.rearrange("(dc p) l -> p dc l", p=128)      # [128, 8, L]

            def proj_dma(lc):
                ls = slice(QC * lc, QC * lc + QC)
                if lc == 0:
                    nc.sync.dma_start(out=wqt_s,
                                      in_=wqt.rearrange("(dc p) m -> p dc m", p=128))
                    nc.sync.dma_start(out=wkvt_s,
                                      in_=wkvt.rearrange("(dc p) m -> p dc m", p=128))
                xbig = xin.tile([128, 8, QC], F16, tag="xt")
                nc.sync.dma_start(out=xbig[:, 0:4, :], in_=xt_r[:, 0:4, ls])
                nc.gpsimd.dma_start(out=xbig[:, 4:8, :], in_=xt_r[:, 4:8, ls])
                tab_eng = nc.scalar if lc == 0 else nc.sync
                tab_eng.dma_start(out=ctab_s[:, ls], in_=ctab[:, ls])
                tab_eng.dma_start(out=s3tab_s[:, ls], in_=s3tab[:, ls])
                xtiles[lc] = xbig

            def proj_slices(lc):
                """Six filler closures computing chunk lc's projections."""
                ls = slice(QC * lc, QC * lc + QC)
                st = {}

                def q1():
                    st["qt_ps"] = mp.tile([128, QC], F32, tag="mp", name=f"qt_ps{lc}")
                    for dc in range(4):
                        nc.tensor.matmul(st["qt_ps"], wqt_s[:, dc, :],
                                         st["x"][:, dc, :],
                                         start=(dc == 0), stop=False)

                def q2():
                    for dc in range(4, 8):
                        nc.tensor.matmul(st["qt_ps"], wqt_s[:, dc, :],
                                         st["x"][:, dc, :],
                                         start=False, stop=(dc == 7))
                    st["qtraw"] = work.tile([128, QC], F16, tag="qtraw", name=f"qtraw{lc}")
                    nc.vector.tensor_copy(st["qtraw"], st["qt_ps"])
                    st["qts"] = work.tile([128, QC], F16, tag="qts", name=f"qts{lc}")
                    for (a, b) in ((0, 32), (32, 0), (64, 96), (96, 64)):
                        nc.gpsimd.dma_start(out=st["qts"][a:a + 32, :],
                                            in_=st["qtraw"][b:b + 32, :])

                def kv1():
                    st["x"] = xtiles.pop(lc)
                    st["kvt_ps"] = mp.tile([128, QC], F32, tag="mp", name=f"kvt_ps{lc}")
                    for dc in range(4):
                        nc.tensor.matmul(st["kvt_ps"], wkvt_s[:, dc, :],
                                         st["x"][:, dc, :],
                                         start=(dc == 0), stop=False)

                def kv2():
                    for dc in range(4, 8):
                        nc.tensor.matmul(st["kvt_ps"], wkvt_s[:, dc, :],
                                         st["x"][:, dc, :],
                                         start=False, stop=(dc == 7))
                    st["kvts"] = work.tile([128, QC], F16, tag="kvts", name=f"kvts{lc}")
                    nc.vector.tensor_copy(st["kvts"], st["kvt_ps"])
                    st["kts"] = work.tile([64, QC], F16, tag="kts", name=f"kts{lc}")
                    nc.gpsimd.dma_start(out=st["kts"][0:32, :],
                                        in_=st["kvts"][32:64, :])
                    nc.gpsimd.dma_start(out=st["kts"][32:64, :],
                                        in_=st["kvts"][0:32, :])

                def krope():
                    t3 = work.tile([64, QC], F16, tag="t1")
                    t4 = work.tile([64, QC], F16, tag="t2")
                    nc.vector.tensor_mul(t3, st["kvts"][0:64, :], ctab_s[0:64, ls])
                    nc.vector.tensor_mul(t4, st["kts"], s3tab_s[0:64, ls])
                    nc.vector.tensor_add(kt2[0:64, ls], t3, t4)
                    nc.gpsimd.dma_start(out=kt2[64:128, ls], in_=kt2[0:64, ls])

                def qrope():
                    t1 = work.tile([128, QC], F16, tag="t1")
                    t2 = work.tile([128, QC], F16, tag="t2")
                    nc.vector.tensor_mul(t1, st["qtraw"], ctab_s[:, ls])
                    nc.vector.tensor_mul(t2, st["qts"], s3tab_s[:, ls])
                    nc.vector.tensor_add(qtrope[:, ls], t1, t2)

                def vt():
                    for t in range(4):
                        vt_ps = mp.tile([128, 64], F16, tag="mp")
                        nc.tensor.transpose(vt_ps,
                                            st["kvts"][64:128, 128 * t:128 * t + 128],
                                            identlo_s[64:128, :])
                        blk = 4 * lc + t
                        nc.vector.tensor_copy(vn[:, 65 * blk:65 * blk + 64], vt_ps)

                return [kv1, kv2, krope, q1, q2, qrope, vt]

            def make_chunk(qc, tail=False):
                qs = slice(QC * qc, QC * qc + QC)
                nkb = 4 * (qc + 1)
                # diagonal k-blocks early: their masks leave the boundary's
                # critical path; block 0 stays first.  The tail chunk instead
                # closes with the diagonals so accumulators finish staggered
                # and the drain overlaps the last score blocks.
                diags = [kb for kb in range(4 * qc, nkb) if kb != 0]
                rest = list(range(1, 4 * qc))
                order = [0] + rest + diags if tail else [0] + diags + rest
                # per q-tile accumulation bracket (first/last kb in `order`)
                first_kb = {}
                last_kb = {}
                for i in range(4):
                    part = [kb for kb in order if kb <= 4 * qc + i]
                    first_kb[i] = part[0]
                    last_kb[i] = part[-1]
                state = {}

                def slot(i, h):
                    t = state["ota"] if i < 2 else state["otb"]
                    return t, 2 * (i % 2) + h

                def qk(kb):
                    ks = slice(KB * kb, KB * kb + KB)
                    m = kb - 4 * qc
                    lo = KB * m if m > 0 else 0
                    st = stp.tile([128, 2, QC], F32, tag="st")
                    qsl = slice(QC * qc + lo, QC * qc + QC)
                    nc.tensor.matmul(st[:, 0, lo:], kt2[0:64, ks],
                                     qtrope[0:64, qsl], start=True, stop=True)
                    nc.tensor.matmul(st[:, 1, lo:], kt2[64:128, ks],
                                     qtrope[64:128, qsl], start=True, stop=True)
                    pt = ptp.tile([128, 2, QC], F16, tag="pt")
                    nc.scalar.activation(pt[:, :, lo:], st[:, :, lo:],
                                         mybir.ActivationFunctionType.Exp,
                                         scale=0.125)
                    if m >= 0:
                        nc.gpsimd.tensor_mul(pt[:, 0, lo:lo + KB],
                                             pt[:, 0, lo:lo + KB], tri_s)
                        nc.gpsimd.tensor_mul(pt[:, 1, lo:lo + KB],
                                             pt[:, 1, lo:lo + KB], tri_s)
                    return pt

                def pv(kb, pt):
                    if "ota" not in state:
                        state["ota"] = otp.tile([128, 4, 128], F32, tag="ot",
                                                name=f"ota{qc}")
                        state["otb"] = otp.tile([128, 4, 128], F32, tag="ot",
                                                name=f"otb{qc}")
                    m = kb - 4 * qc
                    for i in range(max(0, m), 4):
                        for h in (0, 1):
                            t, j = slot(i, h)
                            # start=True marks the whole 2KB PSUM zero region
                            # pending-zero, so only the bank's first write may
                            # set it; other slots' first writes consume the
                            # pending flag (fresh write) with start=False.
                            nc.tensor.matmul(t[:, j, 0:65],
                                             pt[:, h, 128 * i:128 * i + 128],
                                             vn[:, 65 * kb:65 * kb + 65],
                                             start=(kb == first_kb[i] and j == 0),
                                             stop=(kb == last_kb[i]),
                                             skip_group_check=True)

                def finish_a_qtile(i):
                    # normalize: per-q reciprocal of the denominator column,
                    # fused into the PSUM evacuation
                    os_i = osp.tile([128, 128], F16, tag="os")
                    for h in (0, 1):
                        t, j = slot(i, h)
                        rc = work.tile([128, 1], F32, tag="rc")
                        nc.vector.reciprocal(rc, t[:, j, 64:65])
                        if tail and h == 1:
                            nc.scalar.mul(os_i[:, 64 * h:64 * h + 64],
                                          t[:, j, 0:64], rc)
                        else:
                            nc.vector.tensor_scalar_mul(
                                os_i[:, 64 * h:64 * h + 64],
                                t[:, j, 0:64], rc)
                    state.setdefault("oss", {})[i] = os_i

                def finish_a():
                    for i in range(4):
                        finish_a_qtile(i)

                def fb_tr():
                    osts = []
                    for i in range(4):
                        trp = mp.tile([128, 128], F16, tag="mp")
                        nc.tensor.transpose(trp, state["oss"][i], ident_s)
                        ost = osp.tile([128, 128], F16, tag="ost")
                        nc.vector.tensor_copy(ost, trp)
                        osts.append(ost)
                    state["osts"] = osts

                def fb_proj(dcs):
                    def run():
                        for dc in dcs:
                            yps = mp.tile([128, QC], F32, tag="mp")
                            for i in range(4):
                                nc.tensor.matmul(
                                    yps[:, 128 * i:128 * i + 128],
                                    wop_s[:, 128 * dc:128 * dc + 128],
                                    state["osts"][i],
                                    start=True, stop=True,
                                    skip_group_check=True)
                            ysb = work.tile([128, QC], F16, tag="ysb")
                            if tail and dc % 2 == 1:
                                nc.scalar.copy(ysb, yps)
                            else:
                                nc.vector.tensor_copy(ysb, yps)
                            eng = nc.sync if dc % 2 == 0 else nc.gpsimd
                            eng.dma_start(out=yt[128 * dc:128 * dc + 128, qs],
                                          in_=ysb)
                    return run

                def fb_steps():
                    return [fb_tr, fb_proj((0, 1)), fb_proj((2, 3)),
                            fb_proj((4, 5)), fb_proj((6, 7))]

                return nkb, order, qk, pv, finish_a, finish_a_qtile, fb_steps

            nc.sync.dma_start(out=identlo_s, in_=identlo[:, :])
            proj_dma(0)
            # only the denominator-ones columns need initialising; V columns
            # are fully overwritten by the projection pipeline
            nc.gpsimd.memset(
                vn.rearrange("p (b c) -> p b c", c=65)[:, :, 64:65], 1.0)
            nc.scalar.dma_start(out=tri_s, in_=tri[:, :])
            nc.scalar.dma_start(out=ident_s, in_=ident[:, :])
            nc.scalar.dma_start(out=wop_s, in_=wop[:, :])
            for f in proj_slices(0):
                f()
            if LC > 1:
                proj_dma(1)
                for f in proj_slices(1):
                    f()
            if LC > 2:
                proj_dma(2)
            prev = None
            fillers = []
            for qc in range(LC):
                is_tail = qc == LC - 1
                nkb, order, qk, pv, finish_a, finish_a_qt, fb_steps = \
                    make_chunk(qc, tail=is_tail)
                pts = {}
                pts[order[0]] = qk(order[0])
                if nkb > 1:
                    pts[order[1]] = qk(order[1])
                if prev is not None:
                    prev[0]()           # finish_a of previous chunk
                if qc + 3 < LC:
                    proj_dma(qc + 3)
                if qc + 2 < LC:
                    fillers += proj_slices(qc + 2)
                if prev is not None:
                    fillers += prev[1]()
                for i, kb in enumerate(order):
                    if i + 2 < nkb:
                        pts[order[i + 2]] = qk(order[i + 2])
                    pv(kb, pts.pop(kb))
                    # tail: normalize each q-tile as soon as its accumulator
                    # closes (diagonals come last there)
                    if is_tail and kb - 4 * qc >= 0 and i >= nkb - 4:
                        finish_a_qt(kb - 4 * qc)
                    # drain one filler per pv slot; backlog carries across
                    # chunk boundaries so score blocks stay ahead of fillers
                    if fillers:
                        fillers.pop(0)()
                prev = (finish_a, fb_steps)
            while fillers:
                fillers.pop(0)()
            for step in prev[1]():
                step()

    nc.finalize()
    return nc


def prep_inputs(x, Wq, Wk, Wv, Wo, token_positions, L=4096):
    """Host-side sharding + layout prep. Returns per-core input maps."""
    x = np.asarray(x, dtype=np.float32)
    Wq = np.asarray(Wq, dtype=np.float32)
    Wk = np.asarray(Wk, dtype=np.float32)
    Wv = np.asarray(Wv, dtype=np.float32)
    Wo = np.asarray(Wo, dtype=np.float32)
    pos = np.asarray(token_positions)[0].astype(np.float64)

    xt = np.ascontiguousarray(x[0].T).astype(np.float16)   # [D, L]
    i = np.arange(HEAD_DIM // 2, dtype=np.float64)
    freq = THETA ** (-2.0 * i / HEAD_DIM)                  # [32]
    ang = pos[:, None] * freq[None, :]                     # [L, 32]
    cos = np.cos(ang).T
    sin = np.sin(ang).T
    c64 = np.concatenate([cos, cos], axis=0)               # [64, L]
    s64 = np.concatenate([-sin, sin], axis=0)
    ctab = np.ascontiguousarray(np.concatenate([c64, c64], axis=0)).astype(np.float16)
    s3tab = np.ascontiguousarray(np.concatenate([s64, s64], axis=0)).astype(np.float16)

    perm = np.concatenate([np.arange(0, 64, 2), np.arange(1, 64, 2)])
    tri = (np.arange(128)[None, :] >= np.arange(128)[:, None]).astype(np.float16)
    tri = np.ascontiguousarray(tri)
    identlo = np.zeros((128, 64), dtype=np.float16)
    identlo[np.arange(128), np.arange(128) % 64] = 1.0
    ident = np.eye(128, dtype=np.float16)

    in_maps = []
    for c in range(N_CORES):
        h0, h1, g = 2 * c, 2 * c + 1, c // 2
        qrows = np.concatenate([64 * h0 + perm, 64 * h1 + perm])
        wqt = np.ascontiguousarray(Wq[qrows, :].T).astype(np.float16)
        kv = np.concatenate([Wk[64 * g + perm, :], Wv[64 * g:64 * g + 64, :]], axis=0)
        wkvt = np.ascontiguousarray(kv.T).astype(np.float16)
        wop = np.ascontiguousarray(
            np.concatenate([Wo[:, 64 * h0:64 * h0 + 64].T,
                            Wo[:, 64 * h1:64 * h1 + 64].T], axis=0)).astype(np.float16)
        in_maps.append(dict(xt=xt, wqt=wqt, wkvt=wkvt, wop=wop,
                            ctab=ctab, s3tab=s3tab, tri=tri,
                            identlo=identlo, ident=ident))
    return in_maps


_NC_CACHE = {}


def _get_nc(L=4096):
    if L not in _NC_CACHE:
        _NC_CACHE[L] = build_kernel(L)
    return _NC_CACHE[L]


def kernel(x, Wq, Wk, Wv, Wo, token_positions):
    B, L, D = np.asarray(x).shape
    nc = _get_nc(L)
    in_maps = prep_inputs(x, Wq, Wk, Wv, Wo, token_positions, L=L)
    res = run_bass_kernel_spmd(nc, in_maps, list(range(N_CORES)))
    y = np.zeros((D_MODEL, L), dtype=np.float32)
    for r in res.results:
        y += r["yt"].astype(np.float32)
    return np.ascontiguousarray(y.T)[None].astype(np.float32)


# revision 5
# speedup vs baseline: 1.0127x; 1.0127x over previous
"""Trainium2 Bass kernel: GQA multi-head self-attention (B=1, L=4096, D=1024,
16 Q heads, 4 KV heads, head_dim 64, interleaved RoPE, causal softmax).

Sharding: 2 query heads + their (shared) KV head per core, 8 cores.
Each core computes a full-shape partial output Y_c.T = (attn_c @ Wo_c.T).T
(Megatron row-parallel style); the host sums the 8 partials.

Design notes:
  - Scores run as S.T = K @ Q.T ([128 keys, 2 heads x 512 q] PSUM tiles); exp
    runs on the scalar engine straight out of PSUM; diagonal key blocks trim
    the causally-dead columns from both the matmul and the exp.
  - PV uses P as the stationary operand (full 128x128 array) streaming
    [V | 1] blocks, accumulating [q, d] tiles whose 65th column is the
    softmax denominator, so normalize is reciprocal + per-partition
    tensor_scalar multiply fused into the PSUM evacuation.
  - A PSUM zero region is 2KB: only the first matmul into each PV
    accumulator bank sets start=True; other slots' first writes consume the
    bank-wide pending-zero flag.
  - Output projection contracts both heads at once (K=128) after a PE
    transpose of the normalized attention output.
  - The projection pipeline for chunk qc+2 and the finish/out-projection of
    chunk qc-1 are sliced into small "filler" closures interleaved one per
    PV step, keeping the tensor engine fed between exp-paced score blocks.
  - Mask multiplies and SBUF-SBUF swap DMAs ride on gpsimd; big HBM loads
    split across SP/gpsimd/scalar queues; the tail chunk's finish alternates
    PSUM evacuations between the vector and scalar engines.
"""

import sys

for _p in ("/opt/trn_rl_repo",):
    if _p not in sys.path:
        sys.path.insert(0, _p)

import numpy as np

import concourse.bacc as bacc
import concourse.mybir as mybir
import concourse.tile as tile
from concourse.bass_utils import run_bass_kernel_spmd

F32 = mybir.dt.float32
F16 = mybir.dt.float16

D_MODEL = 1024
NUM_HEADS = 16
NUM_KV_HEADS = 4
HEAD_DIM = 64
THETA = 10000.0
N_CORES = 8
QC = 512          # query chunk
KB = 128          # key block


def build_kernel(L=4096):
    """One-core SPMD program. Handles its 2 query heads + 1 shared KV head."""
    nc = bacc.Bacc(None, target_bir_lowering=False)
    LC = L // QC          # number of 512-wide l/q chunks
    NT = L // KB          # number of 128-row key blocks / V tiles

    xt = nc.dram_tensor("xt", [D_MODEL, L], F16, kind="ExternalInput")
    wqt = nc.dram_tensor("wqt", [D_MODEL, 128], F16, kind="ExternalInput")
    wkvt = nc.dram_tensor("wkvt", [D_MODEL, 128], F16, kind="ExternalInput")
    wop = nc.dram_tensor("wop", [128, D_MODEL], F16, kind="ExternalInput")
    ctab = nc.dram_tensor("ctab", [128, L], F16, kind="ExternalInput")
    s3tab = nc.dram_tensor("s3tab", [128, L], F16, kind="ExternalInput")
    tri = nc.dram_tensor("tri", [128, 128], F16, kind="ExternalInput")
    identlo = nc.dram_tensor("identlo", [128, 64], F16, kind="ExternalInput")
    ident = nc.dram_tensor("ident", [128, 128], F16, kind="ExternalInput")
    pq = nc.dram_tensor("pq", [128, 128], F16, kind="ExternalInput")
    pk = nc.dram_tensor("pk", [64, 64], F16, kind="ExternalInput")
    yt = nc.dram_tensor("yt", [D_MODEL, L], F16, kind="ExternalOutput")

    with tile.TileContext(nc) as tc:
        with (
            tc.tile_pool(name="consts", bufs=1) as consts,
            tc.tile_pool(name="big", bufs=1) as big,
            tc.tile_pool(name="xin", bufs=4) as xin,
            tc.tile_pool(name="work", bufs=8) as work,
            tc.tile_pool(name="osp", bufs=16) as osp,
            tc.tile_pool(name="ptp", bufs=6) as ptp,
            tc.tile_pool(name="stp", bufs=2, space="PSUM") as stp,
            tc.tile_pool(name="otp", bufs=2, space="PSUM") as otp,
            tc.tile_pool(name="mp", bufs=2, space="PSUM") as mp,
        ):
            # ---- constants in SBUF ----
            wqt_s = consts.tile([128, 8, 128], F16, tag="wqt")
            wkvt_s = consts.tile([128, 8, 128], F16, tag="wkvt")
            wop_s = consts.tile([128, D_MODEL], F16, tag="wop")
            ctab_s = consts.tile([128, L], F16, tag="ctab")
            s3tab_s = consts.tile([128, L], F16, tag="s3tab")
            tri_s = consts.tile([128, 128], F16, tag="tri")
            identlo_s = consts.tile([128, 64], F16, tag="identlo")
            ident_s = consts.tile([128, 128], F16, tag="ident")
            pq_s = consts.tile([128, 128], F16, tag="pq")
            pk_s = consts.tile([64, 64], F16, tag="pk")

            # ---- persistent per-core activations ----
            qtrope = big.tile([128, L], F16, tag="qtrope")      # [2*64 halfsplit d, L]
            kt2 = big.tile([128, L], F16, tag="kt2")            # K.T duplicated twice
            vn = big.tile([128, NT * 65], F16, tag="vn")        # [V | 1] blocks

            xtiles = {}

            xt_r = xt.rearrange("(dc p) l -> p dc l", p=128)      # [128, 8, L]

            def proj_dma(lc):
                ls = slice(QC * lc, QC * lc + QC)
                xbig = xin.tile([128, 8, QC], F16, tag="xt")
                if lc == 0:
                    nc.sync.dma_start(out=xbig[:, 0:2, :], in_=xt_r[:, 0:2, ls])
                    nc.sync.dma_start(out=xbig[:, 2:4, :], in_=xt_r[:, 2:4, ls])
                    nc.gpsimd.dma_start(out=xbig[:, 4:6, :], in_=xt_r[:, 4:6, ls])
                    nc.gpsimd.dma_start(out=xbig[:, 6:8, :], in_=xt_r[:, 6:8, ls])
                    nc.scalar.dma_start(
                        out=wkvt_s, in_=wkvt.rearrange("(dc p) m -> p dc m", p=128))
                    nc.scalar.dma_start(
                        out=wqt_s, in_=wqt.rearrange("(dc p) m -> p dc m", p=128))
                    nc.scalar.dma_start(out=pk_s, in_=pk[:, :])
                    nc.scalar.dma_start(out=pq_s, in_=pq[:, :])
                    nc.scalar.dma_start(out=ctab_s[:, ls], in_=ctab[:, ls])
                    nc.scalar.dma_start(out=s3tab_s[:, ls], in_=s3tab[:, ls])
                else:
                    nc.sync.dma_start(out=xbig[:, 0:4, :], in_=xt_r[:, 0:4, ls])
                    nc.gpsimd.dma_start(out=xbig[:, 4:8, :], in_=xt_r[:, 4:8, ls])
                    nc.sync.dma_start(out=ctab_s[:, ls], in_=ctab[:, ls])
                    nc.sync.dma_start(out=s3tab_s[:, ls], in_=s3tab[:, ls])
                xtiles[lc] = xbig

            def proj_slices(lc):
                """Six filler closures computing chunk lc's projections."""
                ls = slice(QC * lc, QC * lc + QC)
                st = {}

                def q1():
                    st["qt_ps"] = mp.tile([128, QC], F32, tag="mp", name=f"qt_ps{lc}")
                    for dc in range(4):
                        nc.tensor.matmul(st["qt_ps"], wqt_s[:, dc, :],
                                         st["x"][:, dc, :],
                                         start=(dc == 0), stop=False)

                def q2():
                    for dc in range(4, 8):
                        nc.tensor.matmul(st["qt_ps"], wqt_s[:, dc, :],
                                         st["x"][:, dc, :],
                                         start=False, stop=(dc == 7))
                    st["qtraw"] = work.tile([128, QC], F16, tag="qtraw", name=f"qtraw{lc}")
                    nc.vector.tensor_copy(st["qtraw"], st["qt_ps"])
                    st["qts_ps"] = mp.tile([128, QC], F32, tag="mp",
                                           name=f"qts_ps{lc}")
                    nc.tensor.matmul(st["qts_ps"], pq_s, st["qtraw"],
                                     start=True, stop=True)

                def kv1():
                    st["x"] = xtiles.pop(lc)
                    st["kvt_ps"] = mp.tile([128, QC], F32, tag="mp", name=f"kvt_ps{lc}")
                    for dc in range(4):
                        nc.tensor.matmul(st["kvt_ps"], wkvt_s[:, dc, :],
                                         st["x"][:, dc, :],
                                         start=(dc == 0), stop=False)

                def kv2():
                    for dc in range(4, 8):
                        nc.tensor.matmul(st["kvt_ps"], wkvt_s[:, dc, :],
                                         st["x"][:, dc, :],
                                         start=False, stop=(dc == 7))
                    st["kvts"] = work.tile([128, QC], F16, tag="kvts", name=f"kvts{lc}")
                    nc.vector.tensor_copy(st["kvts"], st["kvt_ps"])
                    st["kts_ps"] = mp.tile([64, QC], F32, tag="mp",
                                           name=f"kts_ps{lc}")
                    nc.tensor.matmul(st["kts_ps"], pk_s, st["kvts"][0:64, :],
                                     start=True, stop=True)

                def krope():
                    t3 = work.tile([64, QC], F16, tag="t1")
                    t4 = work.tile([64, QC], F16, tag="t2")
                    nc.vector.tensor_mul(t3, st["kvts"][0:64, :], ctab_s[0:64, ls])
                    nc.vector.tensor_mul(t4, st["kts_ps"], s3tab_s[0:64, ls])
                    nc.vector.tensor_add(kt2[0:64, ls], t3, t4)
                    nc.gpsimd.dma_start(out=kt2[64:128, ls], in_=kt2[0:64, ls])

                def qrope():
                    t1 = work.tile([128, QC], F16, tag="t1")
                    t2 = work.tile([128, QC], F16, tag="t2")
                    nc.vector.tensor_mul(t1, st["qtraw"], ctab_s[:, ls])
                    nc.vector.tensor_mul(t2, st["qts_ps"], s3tab_s[:, ls])
                    nc.vector.tensor_add(qtrope[:, ls], t1, t2)

                def vt():
                    for t in range(4):
                        vt_ps = mp.tile([128, 64], F16, tag="mp")
                        nc.tensor.transpose(vt_ps,
                                            st["kvts"][64:128, 128 * t:128 * t + 128],
                                            identlo_s[64:128, :])
                        blk = 4 * lc + t
                        nc.vector.tensor_copy(vn[:, 65 * blk:65 * blk + 64], vt_ps)

                return [kv1, kv2, krope, q1, q2, qrope, vt]

            def make_chunk(qc, tail=False):
                qs = slice(QC * qc, QC * qc + QC)
                nkb = 4 * (qc + 1)
                # diagonal k-blocks early: their masks leave the boundary's
                # critical path; block 0 stays first.  The tail chunk instead
                # closes with the diagonals so accumulators finish staggered
                # and the drain overlaps the last score blocks.
                diags = [kb for kb in range(4 * qc, nkb) if kb != 0]
                rest = list(range(1, 4 * qc))
                order = [0] + rest + diags if tail else [0] + diags + rest
                # per q-tile accumulation bracket (first/last kb in `order`)
                first_kb = {}
                last_kb = {}
                for i in range(4):
                    part = [kb for kb in order if kb <= 4 * qc + i]
                    first_kb[i] = part[0]
                    last_kb[i] = part[-1]
                state = {}

                def slot(i, h):
                    t = state["ota"] if i < 2 else state["otb"]
                    return t, 2 * (i % 2) + h

                def qk(kb):
                    ks = slice(KB * kb, KB * kb + KB)
                    m = kb - 4 * qc
                    lo = KB * m if m > 0 else 0
                    st = stp.tile([128, 2, QC], F32, tag="st")
                    qsl = slice(QC * qc + lo, QC * qc + QC)
                    nc.tensor.matmul(st[:, 0, lo:], kt2[0:64, ks],
                                     qtrope[0:64, qsl], start=True, stop=True)
                    nc.tensor.matmul(st[:, 1, lo:], kt2[64:128, ks],
                                     qtrope[64:128, qsl], start=True, stop=True)
                    pt = ptp.tile([128, 2, QC], F16, tag="pt")
                    nc.scalar.activation(pt[:, :, lo:], st[:, :, lo:],
                                         mybir.ActivationFunctionType.Exp,
                                         scale=0.125)
                    if m >= 0:
                        nc.gpsimd.tensor_mul(pt[:, 0, lo:lo + KB],
                                             pt[:, 0, lo:lo + KB], tri_s)
                        nc.gpsimd.tensor_mul(pt[:, 1, lo:lo + KB],
                                             pt[:, 1, lo:lo + KB], tri_s)
                    return pt

                def pv(kb, pt):
                    if "ota" not in state:
                        state["ota"] = otp.tile([128, 4, 128], F32, tag="ot",
                                                name=f"ota{qc}")
                        state["otb"] = otp.tile([128, 4, 128], F32, tag="ot",
                                                name=f"otb{qc}")
                    m = kb - 4 * qc
                    for i in range(max(0, m), 4):
                        for h in (0, 1):
                            t, j = slot(i, h)
                            # start=True marks the whole 2KB PSUM zero region
                            # pending-zero, so only the bank's first write may
                            # set it; other slots' first writes consume the
                            # pending flag (fresh write) with start=False.
                            nc.tensor.matmul(t[:, j, 0:65],
                                             pt[:, h, 128 * i:128 * i + 128],
                                             vn[:, 65 * kb:65 * kb + 65],
                                             start=(kb == first_kb[i] and j == 0),
                                             stop=(kb == last_kb[i]),
                                             skip_group_check=True)

                def finish_a_qtile(i):
                    # normalize: per-q reciprocal of the denominator column,
                    # fused into the PSUM evacuation
                    os_i = osp.tile([128, 128], F16, tag="os")
                    for h in (0, 1):
                        t, j = slot(i, h)
                        rc = work.tile([128, 1], F32, tag="rc")
                        nc.vector.reciprocal(rc, t[:, j, 64:65])
                        if h == 1:
                            nc.scalar.mul(os_i[:, 64 * h:64 * h + 64],
                                          t[:, j, 0:64], rc)
                        else:
                            nc.vector.tensor_scalar_mul(
                                os_i[:, 64 * h:64 * h + 64],
                                t[:, j, 0:64], rc)
                    state.setdefault("oss", {})[i] = os_i

                def finish_a():
                    for i in range(4):
                        finish_a_qtile(i)

                def fb_tr():
                    osts = []
                    for i in range(4):
                        trp = mp.tile([128, 128], F16, tag="mp")
                        nc.tensor.transpose(trp, state["oss"][i], ident_s)
                        ost = osp.tile([128, 128], F16, tag="ost")
                        nc.vector.tensor_copy(ost, trp)
                        osts.append(ost)
                    state["osts"] = osts

                def fb_proj(dcs):
                    def run():
                        for dc in dcs:
                            yps = mp.tile([128, QC], F32, tag="mp")
                            for i in range(4):
                                nc.tensor.matmul(
                                    yps[:, 128 * i:128 * i + 128],
                                    wop_s[:, 128 * dc:128 * dc + 128],
                                    state["osts"][i],
                                    start=True, stop=True,
                                    skip_group_check=True)
                            ysb = work.tile([128, QC], F16, tag="ysb")
                            if tail and dc % 2 == 1:
                                nc.scalar.copy(ysb, yps)
                            else:
                                nc.vector.tensor_copy(ysb, yps)
                            eng = nc.sync if dc % 2 == 0 else nc.gpsimd
                            eng.dma_start(out=yt[128 * dc:128 * dc + 128, qs],
                                          in_=ysb)
                    return run

                def fb_steps():
                    return [fb_tr, fb_proj((0, 1)), fb_proj((2, 3)),
                            fb_proj((4, 5)), fb_proj((6, 7))]

                return nkb, order, qk, pv, finish_a, finish_a_qtile, fb_steps

            nc.sync.dma_start(out=identlo_s, in_=identlo[:, :])
            proj_dma(0)
            # only the denominator-ones columns need initialising; V columns
            # are fully overwritten by the projection pipeline
            nc.gpsimd.memset(
                vn.rearrange("p (b c) -> p b c", c=65)[:, :, 64:65], 1.0)
            nc.scalar.dma_start(out=tri_s, in_=tri[:, :])
            nc.scalar.dma_start(out=ident_s, in_=ident[:, :])
            nc.scalar.dma_start(out=wop_s, in_=wop[:, :])
            for f in proj_slices(0):
                f()
            if LC > 1:
                proj_dma(1)
                for f in proj_slices(1):
                    f()
            if LC > 2:
                proj_dma(2)
            prev = None
            fillers = []
            for qc in range(LC):
                is_tail = qc == LC - 1
                nkb, order, qk, pv, finish_a, finish_a_qt, fb_steps = \
                    make_chunk(qc, tail=is_tail)
                pts = {}
                pts[order[0]] = qk(order[0])
                if nkb > 1:
                    pts[order[1]] = qk(order[1])
                if prev is not None:
                    prev[0]()           # finish_a of previous chunk
                if qc + 3 < LC:
                    proj_dma(qc + 3)
                if qc + 2 < LC:
                    fillers += proj_slices(qc + 2)
                if prev is not None:
                    fillers += prev[1]()
                n0 = len(fillers)
                popped = 0
                for i, kb in enumerate(order):
                    if i + 2 < nkb:
                        pts[order[i + 2]] = qk(order[i + 2])
                    pv(kb, pts.pop(kb))
                    # tail: normalize each q-tile as soon as its accumulator
                    # closes (diagonals come last there)
                    if is_tail and kb - 4 * qc >= 0 and i >= nkb - 4:
                        finish_a_qt(kb - 4 * qc)
                    # drain fillers evenly over the chunk, at most one per pv
                    # slot; backlog carries across chunk boundaries
                    if fillers and (i + 1) * n0 // nkb > popped:
                        fillers.pop(0)()
                        popped += 1
                prev = (finish_a, fb_steps)
            while fillers:
                fillers.pop(0)()
            for step in prev[1]():
                step()

    nc.finalize()
    return nc


def prep_inputs(x, Wq, Wk, Wv, Wo, token_positions, L=4096):
    """Host-side sharding + layout prep. Returns per-core input maps."""
    x = np.asarray(x, dtype=np.float32)
    Wq = np.asarray(Wq, dtype=np.float32)
    Wk = np.asarray(Wk, dtype=np.float32)
    Wv = np.asarray(Wv, dtype=np.float32)
    Wo = np.asarray(Wo, dtype=np.float32)
    pos = np.asarray(token_positions)[0].astype(np.float64)

    xt = np.ascontiguousarray(x[0].T).astype(np.float16)   # [D, L]
    i = np.arange(HEAD_DIM // 2, dtype=np.float64)
    freq = THETA ** (-2.0 * i / HEAD_DIM)                  # [32]
    ang = pos[:, None] * freq[None, :]                     # [L, 32]
    cos = np.cos(ang).T
    sin = np.sin(ang).T
    c64 = np.concatenate([cos, cos], axis=0)               # [64, L]
    s64 = np.concatenate([-sin, sin], axis=0)
    ctab = np.ascontiguousarray(np.concatenate([c64, c64], axis=0)).astype(np.float16)
    s3tab = np.ascontiguousarray(np.concatenate([s64, s64], axis=0)).astype(np.float16)

    perm = np.concatenate([np.arange(0, 64, 2), np.arange(1, 64, 2)])
    tri = (np.arange(128)[None, :] >= np.arange(128)[:, None]).astype(np.float16)
    tri = np.ascontiguousarray(tri)
    identlo = np.zeros((128, 64), dtype=np.float16)
    identlo[np.arange(128), np.arange(128) % 64] = 1.0
    ident = np.eye(128, dtype=np.float16)
    sigq = np.concatenate([np.arange(32, 64), np.arange(0, 32),
                           np.arange(96, 128), np.arange(64, 96)])
    pq = np.zeros((128, 128), dtype=np.float16)
    pq[np.arange(128), sigq] = 1.0
    sigk = np.concatenate([np.arange(32, 64), np.arange(0, 32)])
    pk = np.zeros((64, 64), dtype=np.float16)
    pk[np.arange(64), sigk] = 1.0

    in_maps = []
    for c in range(N_CORES):
        h0, h1, g = 2 * c, 2 * c + 1, c // 2
        qrows = np.concatenate([64 * h0 + perm, 64 * h1 + perm])
        wqt = np.ascontiguousarray(Wq[qrows, :].T).astype(np.float16)
        kv = np.concatenate([Wk[64 * g + perm, :], Wv[64 * g:64 * g + 64, :]], axis=0)
        wkvt = np.ascontiguousarray(kv.T).astype(np.float16)
        wop = np.ascontiguousarray(
            np.concatenate([Wo[:, 64 * h0:64 * h0 + 64].T,
                            Wo[:, 64 * h1:64 * h1 + 64].T], axis=0)).astype(np.float16)
        in_maps.append(dict(xt=xt, wqt=wqt, wkvt=wkvt, wop=wop,
                            ctab=ctab, s3tab=s3tab, tri=tri,
                            identlo=identlo, ident=ident, pq=pq, pk=pk))
    return in_maps


_NC_CACHE = {}


def _get_nc(L=4096):
    if L not in _NC_CACHE:
        _NC_CACHE[L] = build_kernel(L)
    return _NC_CACHE[L]


def kernel(x, Wq, Wk, Wv, Wo, token_positions):
    B, L, D = np.asarray(x).shape
    nc = _get_nc(L)
    in_maps = prep_inputs(x, Wq, Wk, Wv, Wo, token_positions, L=L)
    res = run_bass_kernel_spmd(nc, in_maps, list(range(N_CORES)))
    y = np.zeros((D_MODEL, L), dtype=np.float32)
    for r in res.results:
        y += r["yt"].astype(np.float32)
    return np.ascontiguousarray(y.T)[None].astype(np.float32)


# revision 6
# speedup vs baseline: 1.0231x; 1.0103x over previous
"""Trainium2 Bass kernel: GQA multi-head self-attention (B=1, L=4096, D=1024,
16 Q heads, 4 KV heads, head_dim 64, interleaved RoPE, causal softmax).

Sharding: 2 query heads + their (shared) KV head per core, 8 cores.
Each core computes a full-shape partial output Y_c.T = (attn_c @ Wo_c.T).T
(Megatron row-parallel style); the host sums the 8 partials.

Design notes:
  - Scores run as S.T = K @ Q.T ([128 keys, 2 heads x 512 q] PSUM tiles); exp
    runs on the scalar engine straight out of PSUM; diagonal key blocks trim
    the causally-dead columns from both the matmul and the exp.
  - PV uses P as the stationary operand (full 128x128 array) streaming
    [V | 1] blocks, accumulating [q, d] tiles whose 65th column is the
    softmax denominator, so normalize is reciprocal + per-partition
    tensor_scalar multiply fused into the PSUM evacuation.
  - A PSUM zero region is 2KB: only the first matmul into each PV
    accumulator bank sets start=True; other slots' first writes consume the
    bank-wide pending-zero flag.
  - Output projection contracts both heads at once (K=128) after a PE
    transpose of the normalized attention output.
  - The projection pipeline for chunk qc+2 and the finish/out-projection of
    chunk qc-1 are sliced into small "filler" closures interleaved one per
    PV step, keeping the tensor engine fed between exp-paced score blocks.
  - Mask multiplies and SBUF-SBUF swap DMAs ride on gpsimd; big HBM loads
    split across SP/gpsimd/scalar queues; the tail chunk's finish alternates
    PSUM evacuations between the vector and scalar engines.
"""

import sys

for _p in ("/opt/trn_rl_repo",):
    if _p not in sys.path:
        sys.path.insert(0, _p)

import numpy as np

import concourse.bacc as bacc
import concourse.mybir as mybir
import concourse.tile as tile
from concourse.bass_utils import run_bass_kernel_spmd

F32 = mybir.dt.float32
F16 = mybir.dt.float16

D_MODEL = 1024
NUM_HEADS = 16
NUM_KV_HEADS = 4
HEAD_DIM = 64
THETA = 10000.0
N_CORES = 8
QC = 512          # query chunk
KB = 128          # key block


def build_kernel(L=4096):
    """One-core SPMD program. Handles its 2 query heads + 1 shared KV head."""
    nc = bacc.Bacc(None, target_bir_lowering=False)
    LC = L // QC          # number of 512-wide l/q chunks
    NT = L // KB          # number of 128-row key blocks / V tiles

    xt = nc.dram_tensor("xt", [D_MODEL, L], F16, kind="ExternalInput")
    wqt = nc.dram_tensor("wqt", [D_MODEL, 128], F16, kind="ExternalInput")
    wkvt = nc.dram_tensor("wkvt", [D_MODEL, 128], F16, kind="ExternalInput")
    wop = nc.dram_tensor("wop", [128, D_MODEL], F16, kind="ExternalInput")
    ctab = nc.dram_tensor("ctab", [128, L], F16, kind="ExternalInput")
    s3tab = nc.dram_tensor("s3tab", [128, L], F16, kind="ExternalInput")
    tri = nc.dram_tensor("tri", [128, 128], F16, kind="ExternalInput")
    identlo = nc.dram_tensor("identlo", [128, 64], F16, kind="ExternalInput")
    ident = nc.dram_tensor("ident", [128, 128], F16, kind="ExternalInput")
    pq = nc.dram_tensor("pq", [128, 128], F16, kind="ExternalInput")
    pk = nc.dram_tensor("pk", [64, 64], F16, kind="ExternalInput")
    yt = nc.dram_tensor("yt", [D_MODEL, L], F16, kind="ExternalOutput")

    with tile.TileContext(nc) as tc:
        with (
            tc.tile_pool(name="consts", bufs=1) as consts,
            tc.tile_pool(name="big", bufs=1) as big,
            tc.tile_pool(name="xin", bufs=4) as xin,
            tc.tile_pool(name="work", bufs=8) as work,
            tc.tile_pool(name="osp", bufs=16) as osp,
            tc.tile_pool(name="ptp", bufs=6) as ptp,
            tc.tile_pool(name="stp", bufs=2, space="PSUM") as stp,
            tc.tile_pool(name="otp", bufs=2, space="PSUM") as otp,
            tc.tile_pool(name="mp", bufs=2, space="PSUM") as mp,
        ):
            # ---- constants in SBUF ----
            wqt_s = consts.tile([128, 8, 128], F16, tag="wqt")
            wkvt_s = consts.tile([128, 8, 128], F16, tag="wkvt")
            wop_s = consts.tile([128, D_MODEL], F16, tag="wop")
            ctab_s = consts.tile([128, L], F16, tag="ctab")
            s3tab_s = consts.tile([128, L], F16, tag="s3tab")
            tri_s = consts.tile([128, 128], F16, tag="tri")
            identlo_s = consts.tile([128, 64], F16, tag="identlo")
            ident_s = consts.tile([128, 128], F16, tag="ident")
            pq_s = consts.tile([128, 128], F16, tag="pq")
            pk_s = consts.tile([64, 64], F16, tag="pk")

            # ---- persistent per-core activations ----
            qtrope = big.tile([128, L], F16, tag="qtrope")      # [2*64 halfsplit d, L]
            kt2 = big.tile([128, L], F16, tag="kt2")            # K.T duplicated twice
            vn = big.tile([128, NT * 65], F16, tag="vn")        # [V | 1] blocks

            xtiles = {}

            xt_r = xt.rearrange("(dc p) l -> p dc l", p=128)      # [128, 8, L]

            def proj_dma(lc):
                ls = slice(QC * lc, QC * lc + QC)
                xbig = xin.tile([128, 8, QC], F16, tag="xt")
                if lc == 0:
                    nc.sync.dma_start(out=xbig[:, 0:2, :], in_=xt_r[:, 0:2, ls])
                    nc.sync.dma_start(out=xbig[:, 2:4, :], in_=xt_r[:, 2:4, ls])
                    nc.gpsimd.dma_start(out=xbig[:, 4:6, :], in_=xt_r[:, 4:6, ls])
                    nc.gpsimd.dma_start(out=xbig[:, 6:8, :], in_=xt_r[:, 6:8, ls])
                    nc.scalar.dma_start(
                        out=wkvt_s, in_=wkvt.rearrange("(dc p) m -> p dc m", p=128))
                    nc.scalar.dma_start(
                        out=wqt_s, in_=wqt.rearrange("(dc p) m -> p dc m", p=128))
                    nc.scalar.dma_start(out=pk_s, in_=pk[:, :])
                    nc.scalar.dma_start(out=pq_s, in_=pq[:, :])
                    nc.scalar.dma_start(out=ctab_s[:, ls], in_=ctab[:, ls])
                    nc.scalar.dma_start(out=s3tab_s[:, ls], in_=s3tab[:, ls])
                else:
                    nc.sync.dma_start(out=xbig[:, 0:4, :], in_=xt_r[:, 0:4, ls])
                    nc.gpsimd.dma_start(out=xbig[:, 4:8, :], in_=xt_r[:, 4:8, ls])
                    nc.sync.dma_start(out=ctab_s[:, ls], in_=ctab[:, ls])
                    nc.sync.dma_start(out=s3tab_s[:, ls], in_=s3tab[:, ls])
                xtiles[lc] = xbig

            def proj_slices(lc):
                """Six filler closures computing chunk lc's projections."""
                ls = slice(QC * lc, QC * lc + QC)
                st = {}

                def q1():
                    st["qt_ps"] = mp.tile([128, QC], F32, tag="mp", name=f"qt_ps{lc}")
                    for dc in range(4):
                        nc.tensor.matmul(st["qt_ps"], wqt_s[:, dc, :],
                                         st["x"][:, dc, :],
                                         start=(dc == 0), stop=False)

                def q2():
                    for dc in range(4, 8):
                        nc.tensor.matmul(st["qt_ps"], wqt_s[:, dc, :],
                                         st["x"][:, dc, :],
                                         start=False, stop=(dc == 7))
                    st["qtraw"] = work.tile([128, QC], F16, tag="qtraw", name=f"qtraw{lc}")
                    nc.vector.tensor_copy(st["qtraw"], st["qt_ps"])
                    st["qts_ps"] = mp.tile([128, QC], F32, tag="mp",
                                           name=f"qts_ps{lc}")
                    nc.tensor.matmul(st["qts_ps"], pq_s, st["qtraw"],
                                     start=True, stop=True)

                def kv1():
                    st["x"] = xtiles.pop(lc)
                    st["kvt_ps"] = mp.tile([128, QC], F32, tag="mp", name=f"kvt_ps{lc}")
                    for dc in range(4):
                        nc.tensor.matmul(st["kvt_ps"], wkvt_s[:, dc, :],
                                         st["x"][:, dc, :],
                                         start=(dc == 0), stop=False)

                def kv2():
                    for dc in range(4, 8):
                        nc.tensor.matmul(st["kvt_ps"], wkvt_s[:, dc, :],
                                         st["x"][:, dc, :],
                                         start=False, stop=(dc == 7))
                    st["kvts"] = work.tile([128, QC], F16, tag="kvts", name=f"kvts{lc}")
                    nc.vector.tensor_copy(st["kvts"], st["kvt_ps"])
                    st["kts_ps"] = mp.tile([64, QC], F32, tag="mp",
                                           name=f"kts_ps{lc}")
                    nc.tensor.matmul(st["kts_ps"], pk_s, st["kvts"][0:64, :],
                                     start=True, stop=True)

                def krope():
                    t3 = work.tile([64, QC], F16, tag="t1")
                    t4 = work.tile([64, QC], F16, tag="t2")
                    nc.vector.tensor_mul(t3, st["kvts"][0:64, :], ctab_s[0:64, ls])
                    nc.vector.tensor_mul(t4, st["kts_ps"], s3tab_s[0:64, ls])
                    nc.vector.tensor_add(kt2[0:64, ls], t3, t4)
                    nc.gpsimd.dma_start(out=kt2[64:128, ls], in_=kt2[0:64, ls])

                def qrope():
                    t1 = work.tile([128, QC], F16, tag="t1")
                    t2 = work.tile([128, QC], F16, tag="t2")
                    nc.vector.tensor_mul(t1, st["qtraw"], ctab_s[:, ls])
                    nc.vector.tensor_mul(t2, st["qts_ps"], s3tab_s[:, ls])
                    nc.vector.tensor_add(qtrope[:, ls], t1, t2)

                def vt():
                    for t in range(4):
                        vt_ps = mp.tile([128, 64], F16, tag="mp")
                        nc.tensor.transpose(vt_ps,
                                            st["kvts"][64:128, 128 * t:128 * t + 128],
                                            identlo_s[64:128, :])
                        blk = 4 * lc + t
                        nc.vector.tensor_copy(vn[:, 65 * blk:65 * blk + 64], vt_ps)

                return [kv1, kv2, krope, q1, q2, qrope, vt]

            def make_chunk(qc, tail=False):
                qs = slice(QC * qc, QC * qc + QC)
                nkb = 4 * (qc + 1)
                # diagonal k-blocks early: their masks leave the boundary's
                # critical path; block 0 stays first.  The tail chunk instead
                # closes with the diagonals so accumulators finish staggered
                # and the drain overlaps the last score blocks.
                diags = [kb for kb in range(4 * qc, nkb) if kb != 0]
                rest = list(range(1, 4 * qc))
                order = [0] + rest + diags if tail else [0] + diags + rest
                # per q-tile accumulation bracket (first/last kb in `order`)
                first_kb = {}
                last_kb = {}
                for i in range(4):
                    part = [kb for kb in order if kb <= 4 * qc + i]
                    first_kb[i] = part[0]
                    last_kb[i] = part[-1]
                state = {}

                def slot(i, h):
                    t = state["ota"] if i < 2 else state["otb"]
                    return t, 2 * (i % 2) + h

                def qk(kb):
                    ks = slice(KB * kb, KB * kb + KB)
                    m = kb - 4 * qc
                    lo = KB * m if m > 0 else 0
                    st = stp.tile([128, 2, QC], F32, tag="st")
                    qsl = slice(QC * qc + lo, QC * qc + QC)
                    nc.tensor.matmul(st[:, 0, lo:], kt2[0:64, ks],
                                     qtrope[0:64, qsl], start=True, stop=True)
                    nc.tensor.matmul(st[:, 1, lo:], kt2[64:128, ks],
                                     qtrope[64:128, qsl], start=True, stop=True)
                    pt = ptp.tile([128, 2, QC], F16, tag="pt")
                    nc.scalar.activation(pt[:, :, lo:], st[:, :, lo:],
                                         mybir.ActivationFunctionType.Exp,
                                         scale=0.125)
                    if m >= 0:
                        nc.gpsimd.tensor_mul(pt[:, 0, lo:lo + KB],
                                             pt[:, 0, lo:lo + KB], tri_s)
                        nc.gpsimd.tensor_mul(pt[:, 1, lo:lo + KB],
                                             pt[:, 1, lo:lo + KB], tri_s)
                    return pt

                def pv(kb, pt):
                    if "ota" not in state:
                        state["ota"] = otp.tile([128, 4, 128], F32, tag="ot",
                                                name=f"ota{qc}")
                        state["otb"] = otp.tile([128, 4, 128], F32, tag="ot",
                                                name=f"otb{qc}")
                    m = kb - 4 * qc
                    for i in range(max(0, m), 4):
                        for h in (0, 1):
                            t, j = slot(i, h)
                            # start=True marks the whole 2KB PSUM zero region
                            # pending-zero, so only the bank's first write may
                            # set it; other slots' first writes consume the
                            # pending flag (fresh write) with start=False.
                            nc.tensor.matmul(t[:, j, 0:65],
                                             pt[:, h, 128 * i:128 * i + 128],
                                             vn[:, 65 * kb:65 * kb + 65],
                                             start=(kb == first_kb[i] and j == 0),
                                             stop=(kb == last_kb[i]),
                                             skip_group_check=True)

                def finish_a_qtile(i):
                    # normalize: per-q reciprocal of the denominator column,
                    # fused into the PSUM evacuation
                    os_i = osp.tile([128, 128], F16, tag="os")
                    for h in (0, 1):
                        t, j = slot(i, h)
                        rc = work.tile([128, 1], F32, tag="rc")
                        nc.vector.reciprocal(rc, t[:, j, 64:65])
                        if tail and h == 1:
                            nc.scalar.mul(os_i[:, 64 * h:64 * h + 64],
                                          t[:, j, 0:64], rc)
                        else:
                            nc.vector.tensor_scalar_mul(
                                os_i[:, 64 * h:64 * h + 64],
                                t[:, j, 0:64], rc)
                    state.setdefault("oss", {})[i] = os_i

                def finish_a():
                    for i in range(4):
                        finish_a_qtile(i)

                def fb_tr():
                    osts = []
                    for i in range(4):
                        trp = mp.tile([128, 128], F16, tag="mp")
                        nc.tensor.transpose(trp, state["oss"][i], ident_s)
                        ost = osp.tile([128, 128], F16, tag="ost")
                        nc.vector.tensor_copy(ost, trp)
                        osts.append(ost)
                    state["osts"] = osts

                def fb_proj(dcs):
                    def run():
                        for dc in dcs:
                            yps = mp.tile([128, QC], F32, tag="mp")
                            for i in range(4):
                                nc.tensor.matmul(
                                    yps[:, 128 * i:128 * i + 128],
                                    wop_s[:, 128 * dc:128 * dc + 128],
                                    state["osts"][i],
                                    start=True, stop=True,
                                    skip_group_check=True)
                            ysb = work.tile([128, QC], F16, tag="ysb")
                            if tail and dc % 2 == 1:
                                nc.scalar.copy(ysb, yps)
                            else:
                                nc.vector.tensor_copy(ysb, yps)
                            eng = nc.sync if dc % 2 == 0 else nc.gpsimd
                            eng.dma_start(out=yt[128 * dc:128 * dc + 128, qs],
                                          in_=ysb)
                    return run

                def fb_steps():
                    return [fb_tr, fb_proj((0, 1)), fb_proj((2, 3)),
                            fb_proj((4, 5)), fb_proj((6, 7))]

                return nkb, order, qk, pv, finish_a, finish_a_qtile, fb_steps

            nc.sync.dma_start(out=identlo_s, in_=identlo[:, :])
            proj_dma(0)
            # only the denominator-ones columns need initialising; V columns
            # are fully overwritten by the projection pipeline
            nc.gpsimd.memset(
                vn.rearrange("p (b c) -> p b c", c=65)[:, :, 64:65], 1.0)
            nc.scalar.dma_start(out=tri_s, in_=tri[:, :])
            nc.scalar.dma_start(out=ident_s, in_=ident[:, :])
            nc.scalar.dma_start(out=wop_s, in_=wop[:, :])
            for f in proj_slices(0):
                f()
            if LC > 1:
                proj_dma(1)
                for f in proj_slices(1):
                    f()
            if LC > 2:
                proj_dma(2)
            prev = None
            fillers = []
            for qc in range(LC):
                is_tail = qc == LC - 1
                nkb, order, qk, pv, finish_a, finish_a_qt, fb_steps = \
                    make_chunk(qc, tail=is_tail)
                pts = {}
                pts[order[0]] = qk(order[0])
                if nkb > 1:
                    pts[order[1]] = qk(order[1])
                if prev is not None:
                    prev[0]()           # finish_a of previous chunk
                if qc + 3 < LC:
                    proj_dma(qc + 3)
                if qc + 2 < LC:
                    fillers += proj_slices(qc + 2)
                if prev is not None:
                    fillers += prev[1]()
                n0 = len(fillers)
                popped = 0
                for i, kb in enumerate(order):
                    if i + 2 < nkb:
                        pts[order[i + 2]] = qk(order[i + 2])
                    pv(kb, pts.pop(kb))
                    # tail: normalize each q-tile as soon as its accumulator
                    # closes (diagonals come last there)
                    if is_tail and kb - 4 * qc >= 0 and i >= nkb - 4:
                        finish_a_qt(kb - 4 * qc)
                    # drain fillers evenly over the chunk, at most one per pv
                    # slot; backlog carries across chunk boundaries
                    if fillers and (i + 1) * n0 // nkb > popped:
                        fillers.pop(0)()
                        popped += 1
                prev = (finish_a, fb_steps)
            while fillers:
                fillers.pop(0)()
            for step in prev[1]():
                step()

    nc.finalize()
    return nc


def prep_inputs(x, Wq, Wk, Wv, Wo, token_positions, L=4096):
    """Host-side sharding + layout prep. Returns per-core input maps."""
    x = np.asarray(x, dtype=np.float32)
    Wq = np.asarray(Wq, dtype=np.float32)
    Wk = np.asarray(Wk, dtype=np.float32)
    Wv = np.asarray(Wv, dtype=np.float32)
    Wo = np.asarray(Wo, dtype=np.float32)
    pos = np.asarray(token_positions)[0].astype(np.float64)

    xt = np.ascontiguousarray(x[0].T).astype(np.float16)   # [D, L]
    i = np.arange(HEAD_DIM // 2, dtype=np.float64)
    freq = THETA ** (-2.0 * i / HEAD_DIM)                  # [32]
    ang = pos[:, None] * freq[None, :]                     # [L, 32]
    cos = np.cos(ang).T
    sin = np.sin(ang).T
    c64 = np.concatenate([cos, cos], axis=0)               # [64, L]
    s64 = np.concatenate([-sin, sin], axis=0)
    ctab = np.ascontiguousarray(np.concatenate([c64, c64], axis=0)).astype(np.float16)
    s3tab = np.ascontiguousarray(np.concatenate([s64, s64], axis=0)).astype(np.float16)

    perm = np.concatenate([np.arange(0, 64, 2), np.arange(1, 64, 2)])
    tri = (np.arange(128)[None, :] >= np.arange(128)[:, None]).astype(np.float16)
    tri = np.ascontiguousarray(tri)
    identlo = np.zeros((128, 64), dtype=np.float16)
    identlo[np.arange(128), np.arange(128) % 64] = 1.0
    ident = np.eye(128, dtype=np.float16)
    sigq = np.concatenate([np.arange(32, 64), np.arange(0, 32),
                           np.arange(96, 128), np.arange(64, 96)])
    pq = np.zeros((128, 128), dtype=np.float16)
    pq[np.arange(128), sigq] = 1.0
    sigk = np.concatenate([np.arange(32, 64), np.arange(0, 32)])
    pk = np.zeros((64, 64), dtype=np.float16)
    pk[np.arange(64), sigk] = 1.0

    in_maps = []
    for c in range(N_CORES):
        h0, h1, g = 2 * c, 2 * c + 1, c // 2
        qrows = np.concatenate([64 * h0 + perm, 64 * h1 + perm])
        wqt = np.ascontiguousarray(Wq[qrows, :].T).astype(np.float16)
        kv = np.concatenate([Wk[64 * g + perm, :], Wv[64 * g:64 * g + 64, :]], axis=0)
        wkvt = np.ascontiguousarray(kv.T).astype(np.float16)
        wop = np.ascontiguousarray(
            np.concatenate([Wo[:, 64 * h0:64 * h0 + 64].T,
                            Wo[:, 64 * h1:64 * h1 + 64].T], axis=0)).astype(np.float16)
        in_maps.append(dict(xt=xt, wqt=wqt, wkvt=wkvt, wop=wop,
                            ctab=ctab, s3tab=s3tab, tri=tri,
                            identlo=identlo, ident=ident, pq=pq, pk=pk))
    return in_maps


_NC_CACHE = {}


def _get_nc(L=4096):
    if L not in _NC_CACHE:
        _NC_CACHE[L] = build_kernel(L)
    return _NC_CACHE[L]


def kernel(x, Wq, Wk, Wv, Wo, token_positions):
    B, L, D = np.asarray(x).shape
    nc = _get_nc(L)
    in_maps = prep_inputs(x, Wq, Wk, Wv, Wo, token_positions, L=L)
    res = run_bass_kernel_spmd(nc, in_maps, list(range(N_CORES)))
    y = np.zeros((D_MODEL, L), dtype=np.float32)
    for r in res.results:
        y += r["yt"].astype(np.float32)
    return np.ascontiguousarray(y.T)[None].astype(np.float32)


# revision 7
# speedup vs baseline: 1.0280x; 1.0048x over previous
"""Trainium2 Bass kernel: GQA multi-head self-attention (B=1, L=4096, D=1024,
16 Q heads, 4 KV heads, head_dim 64, interleaved RoPE, causal softmax).

Sharding: 2 query heads + their (shared) KV head per core, 8 cores.
Each core computes a full-shape partial output Y_c.T = (attn_c @ Wo_c.T).T
(Megatron row-parallel style); the host sums the 8 partials.

Design notes:
  - Scores run as S.T = K @ Q.T ([128 keys, 2 heads x 512 q] PSUM tiles); exp
    runs on the scalar engine straight out of PSUM; diagonal key blocks trim
    the causally-dead columns from both the matmul and the exp.
  - PV uses P as the stationary operand (full 128x128 array) streaming
    [V | 1] blocks, accumulating [q, d] tiles whose 65th column is the
    softmax denominator, so normalize is reciprocal + per-partition
    tensor_scalar multiply fused into the PSUM evacuation.
  - A PSUM zero region is 2KB: only the first matmul into each PV
    accumulator bank sets start=True; other slots' first writes consume the
    bank-wide pending-zero flag.
  - Output projection contracts both heads at once (K=128) after a PE
    transpose of the normalized attention output.
  - The projection pipeline for chunk qc+2 and the finish/out-projection of
    chunk qc-1 are sliced into small "filler" closures interleaved one per
    PV step, keeping the tensor engine fed between exp-paced score blocks.
  - Mask multiplies and SBUF-SBUF swap DMAs ride on gpsimd; big HBM loads
    split across SP/gpsimd/scalar queues; the tail chunk's finish alternates
    PSUM evacuations between the vector and scalar engines.
"""

import sys

for _p in ("/opt/trn_rl_repo",):
    if _p not in sys.path:
        sys.path.insert(0, _p)

import numpy as np

import concourse.bacc as bacc
import concourse.mybir as mybir
import concourse.tile as tile
from concourse.bass_utils import run_bass_kernel_spmd

F32 = mybir.dt.float32
F16 = mybir.dt.float16

D_MODEL = 1024
NUM_HEADS = 16
NUM_KV_HEADS = 4
HEAD_DIM = 64
THETA = 10000.0
N_CORES = 8
QC = 512          # query chunk
KB = 128          # key block


def build_kernel(L=4096):
    """One-core SPMD program. Handles its 2 query heads + 1 shared KV head."""
    nc = bacc.Bacc(None, target_bir_lowering=False)
    LC = L // QC          # number of 512-wide l/q chunks
    NT = L // KB          # number of 128-row key blocks / V tiles

    xt = nc.dram_tensor("xt", [D_MODEL, L], F16, kind="ExternalInput")
    wqt = nc.dram_tensor("wqt", [D_MODEL, 128], F16, kind="ExternalInput")
    wkvt = nc.dram_tensor("wkvt", [D_MODEL, 128], F16, kind="ExternalInput")
    wop = nc.dram_tensor("wop", [128, D_MODEL], F16, kind="ExternalInput")
    ctab = nc.dram_tensor("ctab", [128, L], F16, kind="ExternalInput")
    s3tab = nc.dram_tensor("s3tab", [128, L], F16, kind="ExternalInput")
    tri = nc.dram_tensor("tri", [128, 128], F16, kind="ExternalInput")
    identlo = nc.dram_tensor("identlo", [128, 64], F16, kind="ExternalInput")
    ident = nc.dram_tensor("ident", [128, 128], F16, kind="ExternalInput")
    pq = nc.dram_tensor("pq", [128, 128], F16, kind="ExternalInput")
    pk = nc.dram_tensor("pk", [64, 64], F16, kind="ExternalInput")
    yt = nc.dram_tensor("yt", [D_MODEL, L], F16, kind="ExternalOutput")

    with tile.TileContext(nc) as tc:
        with (
            tc.tile_pool(name="consts", bufs=1) as consts,
            tc.tile_pool(name="big", bufs=1) as big,
            tc.tile_pool(name="xin", bufs=4) as xin,
            tc.tile_pool(name="work", bufs=8) as work,
            tc.tile_pool(name="osp", bufs=16) as osp,
            tc.tile_pool(name="ptp", bufs=6) as ptp,
            tc.tile_pool(name="stp", bufs=2, space="PSUM") as stp,
            tc.tile_pool(name="otp", bufs=2, space="PSUM") as otp,
            tc.tile_pool(name="mp", bufs=2, space="PSUM") as mp,
        ):
            # ---- constants in SBUF ----
            wqt_s = consts.tile([128, 8, 128], F16, tag="wqt")
            wkvt_s = consts.tile([128, 8, 128], F16, tag="wkvt")
            wop_s = consts.tile([128, D_MODEL], F16, tag="wop")
            ctab_s = consts.tile([128, L], F16, tag="ctab")
            s3tab_s = consts.tile([128, L], F16, tag="s3tab")
            tri_s = consts.tile([128, 128], F16, tag="tri")
            identlo_s = consts.tile([128, 64], F16, tag="identlo")
            ident_s = consts.tile([128, 128], F16, tag="ident")
            pq_s = consts.tile([128, 128], F16, tag="pq")
            pk_s = consts.tile([64, 64], F16, tag="pk")

            # ---- persistent per-core activations ----
            qtrope = big.tile([128, L], F16, tag="qtrope")      # [2*64 halfsplit d, L]
            kt2 = big.tile([128, L], F16, tag="kt2")            # K.T duplicated twice
            vn = big.tile([128, NT * 65], F16, tag="vn")        # [V | 1] blocks

            xtiles = {}

            xt_r = xt.rearrange("(dc p) l -> p dc l", p=128)      # [128, 8, L]

            def proj_dma(lc):
                ls = slice(QC * lc, QC * lc + QC)
                xbig = xin.tile([128, 8, QC], F16, tag="xt")
                if lc == 0:
                    nc.sync.dma_start(out=xbig[:, 0:2, :], in_=xt_r[:, 0:2, ls])
                    nc.sync.dma_start(out=xbig[:, 2:4, :], in_=xt_r[:, 2:4, ls])
                    nc.gpsimd.dma_start(out=xbig[:, 4:6, :], in_=xt_r[:, 4:6, ls])
                    nc.gpsimd.dma_start(out=xbig[:, 6:8, :], in_=xt_r[:, 6:8, ls])
                    nc.scalar.dma_start(
                        out=wkvt_s, in_=wkvt.rearrange("(dc p) m -> p dc m", p=128))
                    nc.scalar.dma_start(
                        out=wqt_s, in_=wqt.rearrange("(dc p) m -> p dc m", p=128))
                    nc.scalar.dma_start(out=pk_s, in_=pk[:, :])
                    nc.scalar.dma_start(out=pq_s, in_=pq[:, :])
                    nc.scalar.dma_start(out=ctab_s[:, ls], in_=ctab[:, ls])
                    nc.scalar.dma_start(out=s3tab_s[:, ls], in_=s3tab[:, ls])
                else:
                    nc.sync.dma_start(out=xbig[:, 0:4, :], in_=xt_r[:, 0:4, ls])
                    nc.gpsimd.dma_start(out=xbig[:, 4:8, :], in_=xt_r[:, 4:8, ls])
                    nc.sync.dma_start(out=ctab_s[:, ls], in_=ctab[:, ls])
                    nc.sync.dma_start(out=s3tab_s[:, ls], in_=s3tab[:, ls])
                xtiles[lc] = xbig

            def proj_slices(lc):
                """Six filler closures computing chunk lc's projections."""
                ls = slice(QC * lc, QC * lc + QC)
                st = {}

                def q1():
                    st["qt_ps"] = mp.tile([128, QC], F32, tag="mp", name=f"qt_ps{lc}")
                    for dc in range(4):
                        nc.tensor.matmul(st["qt_ps"], wqt_s[:, dc, :],
                                         st["x"][:, dc, :],
                                         start=(dc == 0), stop=False)

                def q2():
                    for dc in range(4, 8):
                        nc.tensor.matmul(st["qt_ps"], wqt_s[:, dc, :],
                                         st["x"][:, dc, :],
                                         start=False, stop=(dc == 7))
                    st["qtraw"] = work.tile([128, QC], F16, tag="qtraw", name=f"qtraw{lc}")
                    nc.vector.tensor_copy(st["qtraw"], st["qt_ps"])
                    if lc <= 1:
                        # startup: row swap via PE permutation skips the
                        # SBUF-SBUF DMA latency on the first chunks' chain
                        st["qts_ps"] = mp.tile([128, QC], F32, tag="mp",
                                               name=f"qts_ps{lc}")
                        nc.tensor.matmul(st["qts_ps"], pq_s, st["qtraw"],
                                         start=True, stop=True)
                    else:
                        st["qts"] = work.tile([128, QC], F16, tag="qts",
                                              name=f"qts{lc}")
                        for (a, b) in ((0, 32), (32, 0), (64, 96), (96, 64)):
                            nc.gpsimd.dma_start(out=st["qts"][a:a + 32, :],
                                                in_=st["qtraw"][b:b + 32, :])

                def kv1():
                    st["x"] = xtiles.pop(lc)
                    st["kvt_ps"] = mp.tile([128, QC], F32, tag="mp", name=f"kvt_ps{lc}")
                    for dc in range(4):
                        nc.tensor.matmul(st["kvt_ps"], wkvt_s[:, dc, :],
                                         st["x"][:, dc, :],
                                         start=(dc == 0), stop=False)

                def kv2():
                    for dc in range(4, 8):
                        nc.tensor.matmul(st["kvt_ps"], wkvt_s[:, dc, :],
                                         st["x"][:, dc, :],
                                         start=False, stop=(dc == 7))
                    st["kvts"] = work.tile([128, QC], F16, tag="kvts", name=f"kvts{lc}")
                    nc.vector.tensor_copy(st["kvts"], st["kvt_ps"])
                    if lc <= 1:
                        st["kts_ps"] = mp.tile([64, QC], F32, tag="mp",
                                               name=f"kts_ps{lc}")
                        nc.tensor.matmul(st["kts_ps"], pk_s, st["kvts"][0:64, :],
                                         start=True, stop=True)
                    else:
                        st["kts"] = work.tile([64, QC], F16, tag="kts",
                                              name=f"kts{lc}")
                        nc.gpsimd.dma_start(out=st["kts"][0:32, :],
                                            in_=st["kvts"][32:64, :])
                        nc.gpsimd.dma_start(out=st["kts"][32:64, :],
                                            in_=st["kvts"][0:32, :])

                def krope():
                    t3 = work.tile([64, QC], F16, tag="t1")
                    t4 = work.tile([64, QC], F16, tag="t2")
                    nc.vector.tensor_mul(t3, st["kvts"][0:64, :], ctab_s[0:64, ls])
                    nc.vector.tensor_mul(
                        t4, st["kts_ps"] if lc <= 1 else st["kts"],
                        s3tab_s[0:64, ls])
                    nc.vector.tensor_add(kt2[0:64, ls], t3, t4)
                    nc.gpsimd.dma_start(out=kt2[64:128, ls], in_=kt2[0:64, ls])

                def qrope():
                    t1 = work.tile([128, QC], F16, tag="t1")
                    t2 = work.tile([128, QC], F16, tag="t2")
                    nc.vector.tensor_mul(t1, st["qtraw"], ctab_s[:, ls])
                    nc.vector.tensor_mul(
                        t2, st["qts_ps"] if lc <= 1 else st["qts"],
                        s3tab_s[:, ls])
                    nc.vector.tensor_add(qtrope[:, ls], t1, t2)

                def vt():
                    for t in range(4):
                        vt_ps = mp.tile([128, 64], F16, tag="mp")
                        nc.tensor.transpose(vt_ps,
                                            st["kvts"][64:128, 128 * t:128 * t + 128],
                                            identlo_s[64:128, :])
                        blk = 4 * lc + t
                        nc.vector.tensor_copy(vn[:, 65 * blk:65 * blk + 64], vt_ps)

                return [kv1, kv2, krope, q1, q2, qrope, vt]

            def make_chunk(qc, tail=False):
                qs = slice(QC * qc, QC * qc + QC)
                nkb = 4 * (qc + 1)
                # diagonal k-blocks early: their masks leave the boundary's
                # critical path; block 0 stays first.  The tail chunk instead
                # closes with the diagonals so accumulators finish staggered
                # and the drain overlaps the last score blocks.
                diags = [kb for kb in range(4 * qc, nkb) if kb != 0]
                rest = list(range(1, 4 * qc))
                order = [0] + rest + diags if tail else [0] + diags + rest
                # per q-tile accumulation bracket (first/last kb in `order`)
                first_kb = {}
                last_kb = {}
                for i in range(4):
                    part = [kb for kb in order if kb <= 4 * qc + i]
                    first_kb[i] = part[0]
                    last_kb[i] = part[-1]
                state = {}

                def slot(i, h):
                    t = state["ota"] if i < 2 else state["otb"]
                    return t, 2 * (i % 2) + h

                def qk(kb):
                    ks = slice(KB * kb, KB * kb + KB)
                    m = kb - 4 * qc
                    lo = KB * m if m > 0 else 0
                    st = stp.tile([128, 2, QC], F32, tag="st")
                    qsl = slice(QC * qc + lo, QC * qc + QC)
                    nc.tensor.matmul(st[:, 0, lo:], kt2[0:64, ks],
                                     qtrope[0:64, qsl], start=True, stop=True)
                    nc.tensor.matmul(st[:, 1, lo:], kt2[64:128, ks],
                                     qtrope[64:128, qsl], start=True, stop=True)
                    pt = ptp.tile([128, 2, QC], F16, tag="pt")
                    nc.scalar.activation(pt[:, :, lo:], st[:, :, lo:],
                                         mybir.ActivationFunctionType.Exp,
                                         scale=0.125)
                    if m >= 0:
                        nc.gpsimd.tensor_mul(pt[:, 0, lo:lo + KB],
                                             pt[:, 0, lo:lo + KB], tri_s)
                        nc.gpsimd.tensor_mul(pt[:, 1, lo:lo + KB],
                                             pt[:, 1, lo:lo + KB], tri_s)
                    return pt

                def pv(kb, pt):
                    if "ota" not in state:
                        state["ota"] = otp.tile([128, 4, 128], F32, tag="ot",
                                                name=f"ota{qc}")
                        state["otb"] = otp.tile([128, 4, 128], F32, tag="ot",
                                                name=f"otb{qc}")
                    m = kb - 4 * qc
                    for i in range(max(0, m), 4):
                        for h in (0, 1):
                            t, j = slot(i, h)
                            # start=True marks the whole 2KB PSUM zero region
                            # pending-zero, so only the bank's first write may
                            # set it; other slots' first writes consume the
                            # pending flag (fresh write) with start=False.
                            nc.tensor.matmul(t[:, j, 0:65],
                                             pt[:, h, 128 * i:128 * i + 128],
                                             vn[:, 65 * kb:65 * kb + 65],
                                             start=(kb == first_kb[i] and j == 0),
                                             stop=(kb == last_kb[i]),
                                             skip_group_check=True)

                def finish_a_qtile(i):
                    # normalize: per-q reciprocal of the denominator column,
                    # fused into the PSUM evacuation
                    os_i = osp.tile([128, 128], F16, tag="os")
                    for h in (0, 1):
                        t, j = slot(i, h)
                        rc = work.tile([128, 1], F32, tag="rc")
                        nc.vector.reciprocal(rc, t[:, j, 64:65])
                        if tail and h == 1:
                            nc.scalar.mul(os_i[:, 64 * h:64 * h + 64],
                                          t[:, j, 0:64], rc)
                        else:
                            nc.vector.tensor_scalar_mul(
                                os_i[:, 64 * h:64 * h + 64],
                                t[:, j, 0:64], rc)
                    state.setdefault("oss", {})[i] = os_i

                def finish_a():
                    for i in range(4):
                        finish_a_qtile(i)

                def fb_tr():
                    osts = []
                    for i in range(4):
                        trp = mp.tile([128, 128], F16, tag="mp")
                        nc.tensor.transpose(trp, state["oss"][i], ident_s)
                        ost = osp.tile([128, 128], F16, tag="ost")
                        nc.vector.tensor_copy(ost, trp)
                        osts.append(ost)
                    state["osts"] = osts

                def fb_proj(dcs):
                    def run():
                        for dc in dcs:
                            yps = mp.tile([128, QC], F32, tag="mp")
                            for i in range(4):
                                nc.tensor.matmul(
                                    yps[:, 128 * i:128 * i + 128],
                                    wop_s[:, 128 * dc:128 * dc + 128],
                                    state["osts"][i],
                                    start=True, stop=True,
                                    skip_group_check=True)
                            ysb = work.tile([128, QC], F16, tag="ysb")
                            if tail and dc % 2 == 1:
                                nc.scalar.copy(ysb, yps)
                            else:
                                nc.vector.tensor_copy(ysb, yps)
                            eng = nc.sync if dc % 2 == 0 else nc.gpsimd
                            eng.dma_start(out=yt[128 * dc:128 * dc + 128, qs],
                                          in_=ysb)
                    return run

                def fb_steps():
                    return [fb_tr, fb_proj((0, 1)), fb_proj((2, 3)),
                            fb_proj((4, 5)), fb_proj((6, 7))]

                return nkb, order, qk, pv, finish_a, finish_a_qtile, fb_steps

            nc.sync.dma_start(out=identlo_s, in_=identlo[:, :])
            proj_dma(0)
            # only the denominator-ones columns need initialising; V columns
            # are fully overwritten by the projection pipeline
            nc.gpsimd.memset(
                vn.rearrange("p (b c) -> p b c", c=65)[:, :, 64:65], 1.0)
            nc.sync.dma_start(out=tri_s, in_=tri[:, :])
            nc.sync.dma_start(out=ident_s, in_=ident[:, :])
            nc.sync.dma_start(out=wop_s, in_=wop[:, :])
            for f in proj_slices(0):
                f()
            if LC > 1:
                proj_dma(1)
                for f in proj_slices(1):
                    f()
            if LC > 2:
                proj_dma(2)
            prev = None
            fillers = []
            for qc in range(LC):
                is_tail = qc == LC - 1
                nkb, order, qk, pv, finish_a, finish_a_qt, fb_steps = \
                    make_chunk(qc, tail=is_tail)
                pts = {}
                pts[order[0]] = qk(order[0])
                if nkb > 1:
                    pts[order[1]] = qk(order[1])
                if prev is not None:
                    prev[0]()           # finish_a of previous chunk
                if qc + 3 < LC:
                    proj_dma(qc + 3)
                if qc + 2 < LC:
                    fillers += proj_slices(qc + 2)
                if prev is not None:
                    fillers += prev[1]()
                n0 = len(fillers)
                popped = 0
                for i, kb in enumerate(order):
                    if i + 2 < nkb:
                        pts[order[i + 2]] = qk(order[i + 2])
                    pv(kb, pts.pop(kb))
                    # tail: normalize each q-tile as soon as its accumulator
                    # closes (diagonals come last there)
                    if is_tail and kb - 4 * qc >= 0 and i >= nkb - 4:
                        finish_a_qt(kb - 4 * qc)
                    # drain fillers evenly over the chunk, at most one per pv
                    # slot; backlog carries across chunk boundaries
                    if fillers and (i + 1) * n0 // nkb > popped:
                        fillers.pop(0)()
                        popped += 1
                prev = (finish_a, fb_steps)
            while fillers:
                fillers.pop(0)()
            for step in prev[1]():
                step()

    nc.finalize()
    return nc


def prep_inputs(x, Wq, Wk, Wv, Wo, token_positions, L=4096):
    """Host-side sharding + layout prep. Returns per-core input maps."""
    x = np.asarray(x, dtype=np.float32)
    Wq = np.asarray(Wq, dtype=np.float32)
    Wk = np.asarray(Wk, dtype=np.float32)
    Wv = np.asarray(Wv, dtype=np.float32)
    Wo = np.asarray(Wo, dtype=np.float32)
    pos = np.asarray(token_positions)[0].astype(np.float64)

    xt = np.ascontiguousarray(x[0].T).astype(np.float16)   # [D, L]
    i = np.arange(HEAD_DIM // 2, dtype=np.float64)
    freq = THETA ** (-2.0 * i / HEAD_DIM)                  # [32]
    ang = pos[:, None] * freq[None, :]                     # [L, 32]
    cos = np.cos(ang).T
    sin = np.sin(ang).T
    c64 = np.concatenate([cos, cos], axis=0)               # [64, L]
    s64 = np.concatenate([-sin, sin], axis=0)
    ctab = np.ascontiguousarray(np.concatenate([c64, c64], axis=0)).astype(np.float16)
    s3tab = np.ascontiguousarray(np.concatenate([s64, s64], axis=0)).astype(np.float16)

    perm = np.concatenate([np.arange(0, 64, 2), np.arange(1, 64, 2)])
    tri = (np.arange(128)[None, :] >= np.arange(128)[:, None]).astype(np.float16)
    tri = np.ascontiguousarray(tri)
    identlo = np.zeros((128, 64), dtype=np.float16)
    identlo[np.arange(128), np.arange(128) % 64] = 1.0
    ident = np.eye(128, dtype=np.float16)
    sigq = np.concatenate([np.arange(32, 64), np.arange(0, 32),
                           np.arange(96, 128), np.arange(64, 96)])
    pq = np.zeros((128, 128), dtype=np.float16)
    pq[np.arange(128), sigq] = 1.0
    sigk = np.concatenate([np.arange(32, 64), np.arange(0, 32)])
    pk = np.zeros((64, 64), dtype=np.float16)
    pk[np.arange(64), sigk] = 1.0

    in_maps = []
    for c in range(N_CORES):
        h0, h1, g = 2 * c, 2 * c + 1, c // 2
        qrows = np.concatenate([64 * h0 + perm, 64 * h1 + perm])
        wqt = np.ascontiguousarray(Wq[qrows, :].T).astype(np.float16)
        kv = np.concatenate([Wk[64 * g + perm, :], Wv[64 * g:64 * g + 64, :]], axis=0)
        wkvt = np.ascontiguousarray(kv.T).astype(np.float16)
        wop = np.ascontiguousarray(
            np.concatenate([Wo[:, 64 * h0:64 * h0 + 64].T,
                            Wo[:, 64 * h1:64 * h1 + 64].T], axis=0)).astype(np.float16)
        in_maps.append(dict(xt=xt, wqt=wqt, wkvt=wkvt, wop=wop,
                            ctab=ctab, s3tab=s3tab, tri=tri,
                            identlo=identlo, ident=ident, pq=pq, pk=pk))
    return in_maps


_NC_CACHE = {}


def _get_nc(L=4096):
    if L not in _NC_CACHE:
        _NC_CACHE[L] = build_kernel(L)
    return _NC_CACHE[L]


def kernel(x, Wq, Wk, Wv, Wo, token_positions):
    B, L, D = np.asarray(x).shape
    nc = _get_nc(L)
    in_maps = prep_inputs(x, Wq, Wk, Wv, Wo, token_positions, L=L)
    res = run_bass_kernel_spmd(nc, in_maps, list(range(N_CORES)))
    y = np.zeros((D_MODEL, L), dtype=np.float32)
    for r in res.results:
        y += r["yt"].astype(np.float32)
    return np.ascontiguousarray(y.T)[None].astype(np.float32)
